# revision 24
# baseline (speedup 1.0000x reference)
"""Trainium2 Bass kernel: per-superpixel mean of CNN features + linear head.

reference computes:
    sums[s, f]  = segment_sum(features, superpixel)      # 1024 segments
    out[s, c]   = (sums[s] / max(count_s, 1)) @ w_node.T # [1024, 21]

v6 design (pure segment-sum on device, bin-sorted pixels):
  * Host-side, each core's 32768 pixels are SORTED by coarse bin
    q = label >> 6 (16 bins) and padded so every bin owns exactly T_BIN
    tiles of 128 pixels (segment-sum is permutation-invariant).  A tile
    is therefore homogeneous in q, so its entire segment contribution is
    ONE matmul
        G_q[r, f] += onehot64[pix, r].T @ feats[pix, f]
    stationary = the tile's fine-label one-hot (64-col LDWEIGHTS),
    moving = the raw [128, 256] bf16 feature tile.  Label s = 64q + r.
  * Two consecutive bins share one PSUM bank ([64, 512]: bin 2k in cols
    0:256, bin 2k+1 in cols 256:512) - safe because bin 2k's values are
    fully written before bin 2k+1's start=True clear, and never written
    again.  8 pairs == 8 banks.
  * One-hots for a whole bin are ONE dual-broadcast DVE is_equal op
    (iota broadcast over tiles, labels broadcast over fine slots);
    64-wide fine labels halve the DVE work vs 128-wide.
  * Features stay in their natural [pix, f] layout (no transposes on
    either host or device); host casts to bf16, halving HBM traffic.
  * Per-pair G is evacuated fp32->bf16 and DMA'd out via the ScalarE
    HWDGE ring as pairs complete (never blocking feats DMAs on the Sync
    ring); the host sums the 8 cores' partials, divides by counts and
    applies the tiny [1024,256]x[256,21] linear head.
"""

import os as _os

import numpy as np
import ml_dtypes

import concourse.mybir as mybir
import concourse.tile as tile
from concourse import bacc
from concourse.bass_utils import run_bass_kernel_spmd

N_CORES = 8
P = 128
F = 256                      # feature dim
NUM_SP = 1024                # superpixel labels
C = 21                       # classes
N_BINS = int(_os.environ.get("KERNEL_NBINS", "8"))   # coarse bins
RFINE = NUM_SP // N_BINS     # fine labels per bin
NPIX = 512 * 512
PIX_PER_CORE = NPIX // N_CORES       # 32768

F32 = mybir.dt.float32
BF16 = mybir.dt.bfloat16


def _build_nc(t_bin):
    chunk_bufs = int(_os.environ.get("KERNEL_CHUNK_BUFS", "8"))
    oh_bufs = int(_os.environ.get("KERNEL_OH_BUFS", "6"))

    n_tiles = N_BINS * t_bin
    chunk_cols = t_bin * F          # one chunk == one coarse bin
    oh_cols = t_bin * RFINE

    nc = bacc.Bacc("TRN2", target_bir_lowering=False)

    feats = nc.dram_tensor("feats", [P, n_tiles, F], BF16, kind="ExternalInput")
    labels = nc.dram_tensor("labels", [P, n_tiles], F32, kind="ExternalInput")
    iota = nc.dram_tensor("iota", [P, RFINE], BF16, kind="ExternalInput")
    out = nc.dram_tensor("out", [N_BINS // 2, RFINE, 2 * F], BF16, kind="ExternalOutput")

    with tile.TileContext(nc) as tc:
        with (
            tc.tile_pool(name="const", bufs=1) as const_pool,
            tc.tile_pool(name="chunk", bufs=chunk_bufs) as chunk_pool,
            tc.tile_pool(name="oh", bufs=oh_bufs) as oh_pool,
            tc.tile_pool(name="evac", bufs=2) as evac_pool,
            tc.tile_pool(name="psum", bufs=N_BINS // 2, space="PSUM") as psum_pool,
        ):
            iota_sb = const_pool.tile([P, RFINE], BF16)
            nc.sync.dma_start(out=iota_sb[:], in_=iota[:])
            labels_sb = const_pool.tile([P, n_tiles], F32)
            nc.sync.dma_start(out=labels_sb[:], in_=labels[:])

            def emit_onehot(q, nparts):
                """one-hots for a whole bin: single dual-broadcast DVE
                is_equal per part (iota bcast over tiles, labels bcast
                over fine slots)."""
                ohb = oh_pool.tile([P, oh_cols], BF16, tag="ohb")
                bnds = [k * t_bin // nparts for k in range(nparts)] + [t_bin]
                for k in range(nparts):
                    lo, hi = bnds[k], bnds[k + 1]
                    tlo, thi = q * t_bin + lo, q * t_bin + hi
                    nc.vector.tensor_tensor(
                        out=ohb[:, lo * RFINE : hi * RFINE].rearrange(
                            "p (t r) -> p t r", t=hi - lo
                        ),
                        in0=iota_sb[:]
                        .unsqueeze(1)
                        .broadcast_to([P, hi - lo, RFINE]),
                        in1=labels_sb[:, tlo:thi]
                        .unsqueeze(2)
                        .broadcast_to([P, thi - tlo, RFINE]),
                        op=mybir.AluOpType.is_equal,
                    )
                return ohb

            def emit_chunk_dma(q, nsub):
                fc = chunk_pool.tile([P, chunk_cols], BF16, tag="fc")
                sb = [k * t_bin // nsub for k in range(nsub)] + [t_bin]
                for k in range(nsub):
                    lo, hi = sb[k], sb[k + 1]
                    nc.sync.dma_start(
                        out=fc[:, lo * F : hi * F],
                        in_=feats[:, q * t_bin + lo : q * t_bin + hi, :],
                    )
                return fc

            gq_live = [None]     # PSUM tile shared by a bin pair

            def emit_bin_mms(q, fc, ohb):
                side = q % 2
                if side == 0:
                    gq_live[0] = psum_pool.tile(
                        [RFINE, 512], F32, tag="gq", name=f"gq{q}"
                    )
                gq = gq_live[0]
                for tb in range(t_bin):
                    nc.tensor.matmul(
                        out=gq[:, side * F : side * F + F],
                        lhsT=ohb[:, tb * RFINE : (tb + 1) * RFINE],
                        rhs=fc[:, tb * F : (tb + 1) * F],
                        start=(tb == 0),
                        stop=(tb == t_bin - 1),
                        skip_group_check=True,
                    )
                # evacuate the pair (fp32 -> bf16) and ship it while
                # later bins still compute; out-DMAs go through the
                # ScalarE HWDGE ring so their sem-waits never block feats
                # DMAs on the Sync ring
                if side == 1:
                    gsb = evac_pool.tile([RFINE, 2 * F], BF16, tag="gsb")
                    nc.scalar.activation(
                        out=gsb[:], in_=gq[:],
                        func=mybir.ActivationFunctionType.Copy,
                    )
                    nc.scalar.dma_start(out=out[q // 2], in_=gsb[:])

            # HAM warm-up: scratch matmuls with no data dependencies keep
            # the PE activity monitor at full clock through the ramp so
            # bin 0's real matmuls start warm
            n_warm = int(_os.environ.get("KERNEL_WARMUP_MMS", "24"))
            if n_warm:
                scratch = acc_scratch = psum_pool.tile(
                    [RFINE, 512], F32, tag="warm", name="warm_ps"
                )
                for k in range(n_warm):
                    nc.tensor.matmul(
                        out=scratch[0:RFINE, 0:RFINE],
                        lhsT=iota_sb[:, 0:RFINE],
                        rhs=iota_sb[:, 0:RFINE],
                        start=True,
                        stop=True,
                        skip_group_check=True,
                    )

            # one-hot production runs ahead of the matmul stream: the
            # first 6 bins' one-hots are emitted upfront (DVE is the
            # early-pipeline bottleneck), the rest one bin ahead
            ohbs = {}
            n_ahead = min(N_BINS, oh_bufs)
            for q in range(n_ahead):
                ohbs[q] = emit_onehot(q, 8 if q == 0 else (2 if q == 1 else 1))
            pending = None
            for q in range(N_BINS):
                nsub = 4 if q in (0, N_BINS - 1) else 1
                fc = emit_chunk_dma(q, nsub)
                ohb = ohbs.pop(q) if q in ohbs else emit_onehot(q, 1)
                if pending is not None:
                    emit_bin_mms(*pending)
                pending = (q, fc, ohb)
            emit_bin_mms(*pending)

    nc.compile()
    return nc


def _install_ntff_hook():
    """Register the axon NTFF profiling hook when the image's antenv
    lacks axon_hooks (mirrors trn_agent_boot._ntff_profile_via_ctypes)."""
    import contextlib
    import ctypes
    import sys
    import types

    if "antenv.axon_hooks" in sys.modules:
        return
    lib = ctypes.CDLL("/opt/axon/libaxon_pjrt.so")
    if not hasattr(lib, "axon_start_nrt_profile"):
        return
    lib.axon_start_nrt_profile.argtypes = [
        ctypes.POINTER(ctypes.c_int64),
        ctypes.c_size_t,
    ]
    lib.axon_start_nrt_profile.restype = ctypes.c_int64
    lib.axon_stop_nrt_profile.argtypes = [ctypes.c_char_p]
    lib.axon_stop_nrt_profile.restype = ctypes.c_int64

    @contextlib.contextmanager
    def _hook(output_dir, device_ids):
        import jax

        jax.devices()
        if device_ids:
            ids = (ctypes.c_int64 * len(device_ids))(*device_ids)
            rc = lib.axon_start_nrt_profile(ids, len(device_ids))
        else:
            rc = lib.axon_start_nrt_profile(None, 0)
        if rc != 0:
            raise RuntimeError(f"axon_start_nrt_profile rc={rc}")
        try:
            yield
        finally:
            n = lib.axon_stop_nrt_profile(str(output_dir).encode())
            print(f"profile: {n} file(s) written to {output_dir}", file=sys.stderr)

    mod = types.ModuleType("antenv.axon_hooks")
    mod.get_axon_ntff_profile_hook = lambda: _hook
    mod.set_axon_ntff_profile_hook = lambda h: None
    sys.modules["antenv.axon_hooks"] = mod


_NC_CACHE = {}


def _get_nc(t_bin):
    if t_bin not in _NC_CACHE:
        _NC_CACHE[t_bin] = _build_nc(t_bin)
    return _NC_CACHE[t_bin]


def kernel(features, superpixel, w_node):
    features = np.asarray(features, dtype=np.float32)
    superpixel = np.asarray(superpixel)
    w_node = np.asarray(w_node, dtype=np.float32)

    feats_flat = features.reshape(NPIX, F)
    sp_flat = superpixel.reshape(NPIX).astype(np.int64)
    shift = int(np.log2(NUM_SP // N_BINS))   # 6 for 16 bins
    fmask = NUM_SP // N_BINS - 1             # 63

    # per-(core, bin) pixel counts decide the padded tile count
    core_sp = sp_flat.reshape(N_CORES, PIX_PER_CORE)
    bin_counts = np.zeros((N_CORES, N_BINS), dtype=np.int64)
    for cidx in range(N_CORES):
        bin_counts[cidx] = np.bincount(core_sp[cidx] >> shift, minlength=N_BINS)
    t_bin = int(-(-bin_counts.max() // P))     # ceil(max/128)
    t_bin = max(8, t_bin)                      # bin-0 one-hot is split 8 ways
    n_tiles = N_BINS * t_bin
    npixp = n_tiles * P

    iota_in = np.broadcast_to(
        np.arange(RFINE, dtype=np.float32)[None, :], (P, RFINE)
    ).astype(ml_dtypes.bfloat16)

    in_maps = []
    for cidx in range(N_CORES):
        base = cidx * PIX_PER_CORE
        sp_c = core_sp[cidx]
        order = np.argsort(sp_c >> shift, kind="stable")
        rows_padded = np.zeros(npixp, dtype=np.int64)   # pads point at row 0
        lab_padded = np.full(npixp, -1.0, dtype=np.float32)
        off = 0
        for b in range(N_BINS):
            n = int(bin_counts[cidx, b])
            seg = order[off : off + n]
            dst = b * t_bin * P
            rows_padded[dst : dst + n] = base + seg
            lab_padded[dst : dst + n] = (sp_c[seg] & fmask).astype(np.float32)
            off += n
        # gather + bf16 cast; pixel (tile t, partition p) at [p, t, :]
        g = feats_flat[rows_padded].astype(ml_dtypes.bfloat16)
        ft = np.ascontiguousarray(g.reshape(n_tiles, P, F).transpose(1, 0, 2))
        lab = np.ascontiguousarray(lab_padded.reshape(n_tiles, P).T)
        in_maps.append({"feats": ft, "labels": lab, "iota": iota_in})

    trace = bool(int(_os.environ.get("KERNEL_TRACE", "0")))
    repeat = int(_os.environ.get("KERNEL_REPEAT", "1"))
    kwargs = {}
    if trace:
        _install_ntff_hook()
        import concourse.bass_utils as _bu

        _bu.upload_artifacts = lambda tmpdir: tmpdir
    base_dir = _os.environ.get("KERNEL_TRACE_DIR") or None
    for rep in range(repeat):
        if trace and base_dir:
            kwargs["tmpdir"] = _os.path.join(base_dir, f"rep{rep}")
            _os.makedirs(kwargs["tmpdir"], exist_ok=True)
        res = run_bass_kernel_spmd(
            _get_nc(t_bin), in_maps, core_ids=list(range(N_CORES)), trace=trace, **kwargs
        )
        if trace:
            print(f"HW exec time: {res.exec_time_ns} ns")
            print(f"profile_json: {res.profile_json}")

    # out[pair][r, side*256+f] -> G[s, f] with s = 64*(2*pair+side) + r
    G = np.zeros((NUM_SP, F), dtype=np.float64)
    for r in res.results:
        o = np.asarray(r["out"], dtype=np.float64)
        o = o.reshape(N_BINS // 2, RFINE, 2, F).transpose(0, 2, 1, 3)
        G += o.reshape(NUM_SP, F)
    counts = np.bincount(sp_flat, minlength=NUM_SP).astype(np.float64)
    node_features = G / np.clip(counts, 1.0, None)[:, None]
    node_potentials = node_features @ w_node.T.astype(np.float64)
    return np.ascontiguousarray(node_potentials).astype(np.float32)


# revision 25
# speedup vs baseline: 1.0039x; 1.0039x over previous
"""Trainium2 Bass kernel: per-superpixel mean of CNN features + linear head.

reference computes:
    sums[s, f]  = segment_sum(features, superpixel)      # 1024 segments
    out[s, c]   = (sums[s] / max(count_s, 1)) @ w_node.T # [1024, 21]

v6 design (pure segment-sum on device, bin-sorted pixels):
  * Host-side, each core's 32768 pixels are SORTED by coarse bin
    q = label >> 6 (16 bins) and padded so every bin owns exactly T_BIN
    tiles of 128 pixels (segment-sum is permutation-invariant).  A tile
    is therefore homogeneous in q, so its entire segment contribution is
    ONE matmul
        G_q[r, f] += onehot64[pix, r].T @ feats[pix, f]
    stationary = the tile's fine-label one-hot (64-col LDWEIGHTS),
    moving = the raw [128, 256] bf16 feature tile.  Label s = 64q + r.
  * Two consecutive bins share one PSUM bank ([64, 512]: bin 2k in cols
    0:256, bin 2k+1 in cols 256:512) - safe because bin 2k's values are
    fully written before bin 2k+1's start=True clear, and never written
    again.  8 pairs == 8 banks.
  * One-hots for a whole bin are ONE dual-broadcast DVE is_equal op
    (iota broadcast over tiles, labels broadcast over fine slots);
    64-wide fine labels halve the DVE work vs 128-wide.
  * Features stay in their natural [pix, f] layout (no transposes on
    either host or device); host casts to bf16, halving HBM traffic.
  * Per-pair G is evacuated fp32->bf16 and DMA'd out via the ScalarE
    HWDGE ring as pairs complete (never blocking feats DMAs on the Sync
    ring); the host sums the 8 cores' partials, divides by counts and
    applies the tiny [1024,256]x[256,21] linear head.
"""

import os as _os

import numpy as np
import ml_dtypes

import concourse.mybir as mybir
import concourse.tile as tile
from concourse import bacc
from concourse.bass_utils import run_bass_kernel_spmd

N_CORES = 8
P = 128
F = 256                      # feature dim
NUM_SP = 1024                # superpixel labels
C = 21                       # classes
N_BINS = int(_os.environ.get("KERNEL_NBINS", "8"))   # coarse bins
RFINE = NUM_SP // N_BINS     # fine labels per bin
NPIX = 512 * 512
PIX_PER_CORE = NPIX // N_CORES       # 32768

F32 = mybir.dt.float32
BF16 = mybir.dt.bfloat16


def _build_nc(t_bin):
    chunk_bufs = int(_os.environ.get("KERNEL_CHUNK_BUFS", "8"))
    oh_bufs = int(_os.environ.get("KERNEL_OH_BUFS", "6"))

    n_tiles = N_BINS * t_bin
    chunk_cols = t_bin * F          # one chunk == one coarse bin
    oh_cols = t_bin * RFINE

    nc = bacc.Bacc("TRN2", target_bir_lowering=False)

    feats = nc.dram_tensor("feats", [P, n_tiles, F], BF16, kind="ExternalInput")
    labels = nc.dram_tensor("labels", [P, n_tiles], F32, kind="ExternalInput")
    iota = nc.dram_tensor("iota", [P, RFINE], BF16, kind="ExternalInput")
    out = nc.dram_tensor("out", [RFINE, N_BINS * F], BF16, kind="ExternalOutput")

    with tile.TileContext(nc) as tc:
        with (
            tc.tile_pool(name="const", bufs=1) as const_pool,
            tc.tile_pool(name="chunk", bufs=chunk_bufs) as chunk_pool,
            tc.tile_pool(name="oh", bufs=oh_bufs) as oh_pool,
            tc.tile_pool(name="psum", bufs=N_BINS // 2, space="PSUM") as psum_pool,
        ):
            iota_sb = const_pool.tile([P, RFINE], BF16)
            nc.sync.dma_start(out=iota_sb[:], in_=iota[:])
            labels_sb = const_pool.tile([P, n_tiles], F32)
            nc.sync.dma_start(out=labels_sb[:], in_=labels[:])
            # all pair evacs land here; ONE out-DMA ships it at the end so
            # no mid-stream DMA ever waits on compute (the 8 DMA sem lanes
            # are recycled round-robin - a compute-gated out-DMA in the
            # middle head-blocks later feats DMAs on its lane)
            out_sb = const_pool.tile([RFINE, N_BINS * F], BF16)

            def emit_onehot(q, nparts):
                """one-hots for a whole bin: single dual-broadcast DVE
                is_equal per part (iota bcast over tiles, labels bcast
                over fine slots)."""
                ohb = oh_pool.tile([P, oh_cols], BF16, tag="ohb")
                bnds = [k * t_bin // nparts for k in range(nparts)] + [t_bin]
                for k in range(nparts):
                    lo, hi = bnds[k], bnds[k + 1]
                    tlo, thi = q * t_bin + lo, q * t_bin + hi
                    nc.vector.tensor_tensor(
                        out=ohb[:, lo * RFINE : hi * RFINE].rearrange(
                            "p (t r) -> p t r", t=hi - lo
                        ),
                        in0=iota_sb[:]
                        .unsqueeze(1)
                        .broadcast_to([P, hi - lo, RFINE]),
                        in1=labels_sb[:, tlo:thi]
                        .unsqueeze(2)
                        .broadcast_to([P, thi - tlo, RFINE]),
                        op=mybir.AluOpType.is_equal,
                    )
                return ohb

            def emit_chunk_dma(q, nsub):
                fc = chunk_pool.tile([P, chunk_cols], BF16, tag="fc")
                sb = [k * t_bin // nsub for k in range(nsub)] + [t_bin]
                for k in range(nsub):
                    lo, hi = sb[k], sb[k + 1]
                    nc.sync.dma_start(
                        out=fc[:, lo * F : hi * F],
                        in_=feats[:, q * t_bin + lo : q * t_bin + hi, :],
                    )
                return fc

            gq_live = [None]     # PSUM tile shared by a bin pair

            def emit_bin_mms(q, fc, ohb):
                side = q % 2
                if side == 0:
                    gq_live[0] = psum_pool.tile(
                        [RFINE, 512], F32, tag="gq", name=f"gq{q}"
                    )
                gq = gq_live[0]
                for tb in range(t_bin):
                    nc.tensor.matmul(
                        out=gq[:, side * F : side * F + F],
                        lhsT=ohb[:, tb * RFINE : (tb + 1) * RFINE],
                        rhs=fc[:, tb * F : (tb + 1) * F],
                        start=(tb == 0),
                        stop=(tb == t_bin - 1),
                        skip_group_check=True,
                    )
                # evacuate the pair (fp32 -> bf16) into the staging tile
                # while later bins still compute
                if side == 1:
                    nc.scalar.activation(
                        out=out_sb[:, (q - 1) * F : (q + 1) * F], in_=gq[:],
                        func=mybir.ActivationFunctionType.Copy,
                    )

            # HAM warm-up: scratch matmuls with no data dependencies keep
            # the PE activity monitor at full clock through the ramp so
            # bin 0's real matmuls start warm
            n_warm = int(_os.environ.get("KERNEL_WARMUP_MMS", "24"))
            if n_warm:
                scratch = acc_scratch = psum_pool.tile(
                    [RFINE, 512], F32, tag="warm", name="warm_ps"
                )
                for k in range(n_warm):
                    nc.tensor.matmul(
                        out=scratch[0:RFINE, 0:RFINE],
                        lhsT=iota_sb[:, 0:RFINE],
                        rhs=iota_sb[:, 0:RFINE],
                        start=True,
                        stop=True,
                        skip_group_check=True,
                    )

            # one-hot production runs ahead of the matmul stream: the
            # first 6 bins' one-hots are emitted upfront (DVE is the
            # early-pipeline bottleneck), the rest one bin ahead
            ohbs = {}
            n_ahead = min(N_BINS, oh_bufs)
            for q in range(n_ahead):
                ohbs[q] = emit_onehot(q, 8 if q == 0 else (2 if q == 1 else 1))
            pending = None
            for q in range(N_BINS):
                nsub = 2 if q in (0, N_BINS - 1) else 1
                fc = emit_chunk_dma(q, nsub)
                ohb = ohbs.pop(q) if q in ohbs else emit_onehot(q, 1)
                if pending is not None:
                    emit_bin_mms(*pending)
                pending = (q, fc, ohb)
            emit_bin_mms(*pending)
            nc.sync.dma_start(out=out[:], in_=out_sb[:])

    nc.compile()
    return nc


def _install_ntff_hook():
    """Register the axon NTFF profiling hook when the image's antenv
    lacks axon_hooks (mirrors trn_agent_boot._ntff_profile_via_ctypes)."""
    import contextlib
    import ctypes
    import sys
    import types

    if "antenv.axon_hooks" in sys.modules:
        return
    lib = ctypes.CDLL("/opt/axon/libaxon_pjrt.so")
    if not hasattr(lib, "axon_start_nrt_profile"):
        return
    lib.axon_start_nrt_profile.argtypes = [
        ctypes.POINTER(ctypes.c_int64),
        ctypes.c_size_t,
    ]
    lib.axon_start_nrt_profile.restype = ctypes.c_int64
    lib.axon_stop_nrt_profile.argtypes = [ctypes.c_char_p]
    lib.axon_stop_nrt_profile.restype = ctypes.c_int64

    @contextlib.contextmanager
    def _hook(output_dir, device_ids):
        import jax

        jax.devices()
        if device_ids:
            ids = (ctypes.c_int64 * len(device_ids))(*device_ids)
            rc = lib.axon_start_nrt_profile(ids, len(device_ids))
        else:
            rc = lib.axon_start_nrt_profile(None, 0)
        if rc != 0:
            raise RuntimeError(f"axon_start_nrt_profile rc={rc}")
        try:
            yield
        finally:
            n = lib.axon_stop_nrt_profile(str(output_dir).encode())
            print(f"profile: {n} file(s) written to {output_dir}", file=sys.stderr)

    mod = types.ModuleType("antenv.axon_hooks")
    mod.get_axon_ntff_profile_hook = lambda: _hook
    mod.set_axon_ntff_profile_hook = lambda h: None
    sys.modules["antenv.axon_hooks"] = mod


_NC_CACHE = {}


def _get_nc(t_bin):
    if t_bin not in _NC_CACHE:
        _NC_CACHE[t_bin] = _build_nc(t_bin)
    return _NC_CACHE[t_bin]


def kernel(features, superpixel, w_node):
    features = np.asarray(features, dtype=np.float32)
    superpixel = np.asarray(superpixel)
    w_node = np.asarray(w_node, dtype=np.float32)

    feats_flat = features.reshape(NPIX, F)
    sp_flat = superpixel.reshape(NPIX).astype(np.int64)
    shift = int(np.log2(NUM_SP // N_BINS))   # 6 for 16 bins
    fmask = NUM_SP // N_BINS - 1             # 63

    # per-(core, bin) pixel counts decide the padded tile count
    core_sp = sp_flat.reshape(N_CORES, PIX_PER_CORE)
    bin_counts = np.zeros((N_CORES, N_BINS), dtype=np.int64)
    for cidx in range(N_CORES):
        bin_counts[cidx] = np.bincount(core_sp[cidx] >> shift, minlength=N_BINS)
    t_bin = int(-(-bin_counts.max() // P))     # ceil(max/128)
    t_bin = max(8, t_bin)                      # bin-0 one-hot is split 8 ways
    n_tiles = N_BINS * t_bin
    npixp = n_tiles * P

    iota_in = np.broadcast_to(
        np.arange(RFINE, dtype=np.float32)[None, :], (P, RFINE)
    ).astype(ml_dtypes.bfloat16)

    in_maps = []
    for cidx in range(N_CORES):
        base = cidx * PIX_PER_CORE
        sp_c = core_sp[cidx]
        order = np.argsort(sp_c >> shift, kind="stable")
        rows_padded = np.zeros(npixp, dtype=np.int64)   # pads point at row 0
        lab_padded = np.full(npixp, -1.0, dtype=np.float32)
        off = 0
        for b in range(N_BINS):
            n = int(bin_counts[cidx, b])
            seg = order[off : off + n]
            dst = b * t_bin * P
            rows_padded[dst : dst + n] = base + seg
            lab_padded[dst : dst + n] = (sp_c[seg] & fmask).astype(np.float32)
            off += n
        # gather + bf16 cast; pixel (tile t, partition p) at [p, t, :]
        g = feats_flat[rows_padded].astype(ml_dtypes.bfloat16)
        ft = np.ascontiguousarray(g.reshape(n_tiles, P, F).transpose(1, 0, 2))
        lab = np.ascontiguousarray(lab_padded.reshape(n_tiles, P).T)
        in_maps.append({"feats": ft, "labels": lab, "iota": iota_in})

    trace = bool(int(_os.environ.get("KERNEL_TRACE", "0")))
    repeat = int(_os.environ.get("KERNEL_REPEAT", "1"))
    kwargs = {}
    if trace:
        _install_ntff_hook()
        import concourse.bass_utils as _bu

        _bu.upload_artifacts = lambda tmpdir: tmpdir
    base_dir = _os.environ.get("KERNEL_TRACE_DIR") or None
    for rep in range(repeat):
        if trace and base_dir:
            kwargs["tmpdir"] = _os.path.join(base_dir, f"rep{rep}")
            _os.makedirs(kwargs["tmpdir"], exist_ok=True)
        res = run_bass_kernel_spmd(
            _get_nc(t_bin), in_maps, core_ids=list(range(N_CORES)), trace=trace, **kwargs
        )
        if trace:
            print(f"HW exec time: {res.exec_time_ns} ns")
            print(f"profile_json: {res.profile_json}")

    # out[r, (pair, side, f)] -> G[s, f] with s = RFINE*(2*pair+side) + r
    G = np.zeros((NUM_SP, F), dtype=np.float64)
    for r in res.results:
        o = np.asarray(r["out"], dtype=np.float64)
        o = o.reshape(RFINE, N_BINS // 2, 2, F).transpose(1, 2, 0, 3)
        G += o.reshape(NUM_SP, F)
    counts = np.bincount(sp_flat, minlength=NUM_SP).astype(np.float64)
    node_features = G / np.clip(counts, 1.0, None)[:, None]
    node_potentials = node_features @ w_node.T.astype(np.float64)
    return np.ascontiguousarray(node_potentials).astype(np.float32)
